# revision 5
# baseline (speedup 1.0000x reference)
"""Trainium2 Bass kernel for nn_ConcatHeadModule (pairwise MLP scores).

scores[i, j] = W_out . tanh(th[i] + tm[j] + hid2_bias) + out_bias
  th = tanh(xf @ W_foh + cat_bias[:H]) @ W_hid2[:H]
  tm = tanh(xf @ W_fom + cat_bias[H:]) @ W_hid2[H:]

Sharding: rows i split across 8 cores (128 rows each); everything else
replicated. Layout on device: hid2 (64) stacked twice on partitions so each
tanh/reduce step covers a pair of output rows (i, i+64) at once.
"""

import sys

sys.path.insert(0, "/opt/trn_rl_repo")

import numpy as np

import concourse.bass as bass
import concourse.tile as tile
from concourse import bacc, mybir
from concourse.bass_utils import run_bass_kernel_spmd

N = 1024          # nodes
F = 512           # 2 * LDIMS
H = 128           # hidden
D = 64            # hid2
NCORES = 8
R = N // NCORES   # rows per core = 128
NPAIR = R // 2    # row pairs per core = 64

F32 = mybir.dt.float32
Tanh = mybir.ActivationFunctionType.Tanh

# pairs handled per ACT block (tanh over [128, PAIRS_PER_BLOCK*1024])
PAIRS_PER_BLOCK = 4
# pairs per PSUM group: 4 col-groups x 2 bank-pairs
PAIRS_PER_GROUP = 8
BLOCKS_PER_GROUP = PAIRS_PER_GROUP // PAIRS_PER_BLOCK
NGROUPS = NPAIR // PAIRS_PER_GROUP


def _build_program(out_bias: float):
    nc = bacc.Bacc("TRN2", target_bir_lowering=False, debug=False,
                   num_devices=NCORES)

    xt_d = nc.dram_tensor("xt", [F, N], F32, kind="ExternalInput")
    xtm_d = nc.dram_tensor("xtm", [F, R], F32, kind="ExternalInput")
    wfoh_d = nc.dram_tensor("wfoh", [F, H], F32, kind="ExternalInput")
    wfom_d = nc.dram_tensor("wfom", [F, H], F32, kind="ExternalInput")
    cbh_d = nc.dram_tensor("cbh", [H, 1], F32, kind="ExternalInput")
    cbm_d = nc.dram_tensor("cbm", [H, 1], F32, kind="ExternalInput")
    h2b2_d = nc.dram_tensor("h2b2", [2 * D, 1], F32, kind="ExternalInput")
    w2_d = nc.dram_tensor("w2", [2 * D, 32], F32, kind="ExternalInput")
    out_d = nc.dram_tensor("out", [R, N], F32, kind="ExternalOutput")

    with tile.TileContext(nc) as tc:
        with (
            tc.tile_pool(name="consts", bufs=1) as consts,
            tc.tile_pool(name="proj", bufs=1) as proj,
            tc.tile_pool(name="addb", bufs=2) as addp,
            tc.tile_pool(name="tanb", bufs=2) as tanp,
            tc.tile_pool(name="stage", bufs=2) as stagep,
            tc.tile_pool(name="ps", bufs=2, space="PSUM") as psum,
        ):
            # ---- load constants / inputs ----
            xtb = []
            for q in range(4):
                t = consts.tile([H, N], F32, tag=f"xtb{q}")
                nc.sync.dma_start(t[:], xt_d[q * H:(q + 1) * H, :])
                xtb.append(t)
            xtm = []
            for q in range(4):
                t = consts.tile([H, R], F32, tag=f"xtm{q}")
                nc.sync.dma_start(t[:], xtm_d[q * H:(q + 1) * H, :])
                xtm.append(t)
            wfoh = []
            wfom = []
            for q in range(4):
                t = consts.tile([H, H], F32, tag=f"wfoh{q}")
                nc.sync.dma_start(t[:], wfoh_d[q * H:(q + 1) * H, :])
                wfoh.append(t)
                t = consts.tile([H, H], F32, tag=f"wfom{q}")
                nc.sync.dma_start(t[:], wfom_d[q * H:(q + 1) * H, :])
                wfom.append(t)
            cbh = consts.tile([H, 1], F32, tag="cbh")
            nc.sync.dma_start(cbh[:], cbh_d[:])
            cbm = consts.tile([H, 1], F32, tag="cbm")
            nc.sync.dma_start(cbm[:], cbm_d[:])
            h2b2 = consts.tile([2 * D, 1], F32, tag="h2b2")
            nc.sync.dma_start(h2b2[:], h2b2_d[:])
            w2 = consts.tile([2 * D, 32], F32, tag="w2")
            nc.sync.dma_start(w2[:], w2_d[:])

            # ---- projections ----
            # modfovT over all nodes: tanh(W_fom^T @ xf^T + cbm)  [H, N]
            tanhm = proj.tile([H, N], F32, tag="tanhm")
            for jh in range(2):
                pm = psum.tile([H, 512], F32, tag="ps")
                mv = slice(jh * 512, (jh + 1) * 512)
                for q in range(4):
                    nc.tensor.matmul(pm[:], wfom[q][:], xtb[q][:, mv],
                                     start=(q == 0), stop=(q == 3))
                nc.scalar.activation(tanhm[:, mv], pm[:], Tanh, bias=cbm[:])
            # headfovT for this core's rows: [H, R]
            tanhh = proj.tile([H, R], F32, tag="tanhh")
            pm2 = psum.tile([H, R], F32, tag="ps")
            for q in range(4):
                nc.tensor.matmul(pm2[:], wfoh[q][:], xtm[q][:],
                                 start=(q == 0), stop=(q == 3))
            nc.scalar.activation(tanhh[:], pm2[:], Tanh, bias=cbh[:])

            # load W_hid2 halves
            wh2t = consts.tile([H, D], F32, tag="wh2t")
            wh2b = consts.tile([H, D], F32, tag="wh2b")
            wh2t_d = nc.dram_tensor("wh2t", [H, D], F32, kind="ExternalInput")
            wh2b_d = nc.dram_tensor("wh2b", [H, D], F32, kind="ExternalInput")
            nc.sync.dma_start(wh2t[:], wh2t_d[:])
            nc.sync.dma_start(wh2b[:], wh2b_d[:])

            # tm_tile[d + 64*half, j] = tmT[d, j] + hid2_bias[d], both halves
            tm_tile = proj.tile([2 * D, N], F32, tag="tm_tile")
            pt = psum.tile([2 * D, N], F32, tag="ps")
            for o in (0, D):
                for jh in range(2):
                    mv = slice(jh * 512, (jh + 1) * 512)
                    nc.tensor.matmul(pt[o:o + D, mv], wh2b[:], tanhm[:, mv],
                                     start=True, stop=True)
            nc.vector.tensor_scalar_add(tm_tile[:], pt[:], h2b2[:])

            # th_stack[d + 64*half, p] = thT[d, p + 64*half]   [128, 64]
            th_stack = proj.tile([2 * D, NPAIR], F32, tag="th_stack")
            ps3 = psum.tile([2 * D, NPAIR], F32, tag="ps")
            nc.tensor.matmul(ps3[0:D, :], wh2t[:], tanhh[:, 0:NPAIR],
                             start=True, stop=True)
            nc.tensor.matmul(ps3[D:2 * D, :], wh2t[:], tanhh[:, NPAIR:R],
                             start=True, stop=True)
            nc.vector.tensor_copy(th_stack[:], ps3[:])

            # ---- main pair loop ----
            # HBM view: row = 64*w + 8*g + 2*v + b  ->  [g][v][w, b, j]
            out_view = out_d[:].rearrange(
                "(w g v b) j -> g v w b j", w=2, g=NGROUPS, v=4, b=2)
            for g in range(NGROUPS):
                pscore = psum.tile([128, 2048], F32, tag="ps")
                for blk in range(BLOCKS_PER_GROUP):
                    addb = addp.tile([128, PAIRS_PER_BLOCK * N], F32,
                                     tag="addb")
                    tanb = tanp.tile([128, PAIRS_PER_BLOCK * N], F32,
                                     tag="tanb")
                    for u in range(PAIRS_PER_BLOCK):
                        p = g * PAIRS_PER_GROUP + blk * PAIRS_PER_BLOCK + u
                        nc.vector.tensor_scalar_add(
                            addb[:, u * N:(u + 1) * N], tm_tile[:],
                            th_stack[:, p:p + 1])
                    nc.scalar.activation(tanb[:], addb[:], Tanh)
                    for u in range(PAIRS_PER_BLOCK):
                        q = blk * PAIRS_PER_BLOCK + u
                        v, b = divmod(q, 2)
                        for jh in range(2):
                            nc.tensor.matmul(
                                pscore[32 * v:32 * v + 32,
                                       b * N + jh * 512:b * N + (jh + 1) * 512],
                                w2[:],
                                tanb[:, u * N + jh * 512:u * N + (jh + 1) * 512],
                                start=True, stop=True,
                                tile_position=(0, 32 * v))
                stg = stagep.tile([128, 2048], F32, tag="stg")
                nc.vector.tensor_scalar_add(stg[:], pscore[:], out_bias)
                for v in range(4):
                    src = stg[32 * v:32 * v + 2, :].rearrange(
                        "w (b j) -> w b j", b=2)
                    nc.sync.dma_start(out_view[g, v], src)

    nc.compile()
    return nc


def kernel(x, W_foh, W_fom, cat_bias, W_hid2, hid2_bias, W_out, out_bias):
    x = np.asarray(x, dtype=np.float32)
    W_foh = np.asarray(W_foh, dtype=np.float32)
    W_fom = np.asarray(W_fom, dtype=np.float32)
    cat_bias = np.asarray(cat_bias, dtype=np.float32)
    W_hid2 = np.asarray(W_hid2, dtype=np.float32)
    hid2_bias = np.asarray(hid2_bias, dtype=np.float32)
    W_out = np.asarray(W_out, dtype=np.float32)
    out_bias = np.asarray(out_bias, dtype=np.float32)

    xf = x.reshape(N, F)
    xt = np.ascontiguousarray(xf.T)                      # [F, N]
    cbh = np.ascontiguousarray(cat_bias[:H].reshape(H, 1))
    cbm = np.ascontiguousarray(cat_bias[H:].reshape(H, 1))
    h2b2 = np.ascontiguousarray(
        np.concatenate([hid2_bias, hid2_bias]).reshape(2 * D, 1))
    w2 = np.zeros((2 * D, 32), dtype=np.float32)
    w2[:D, 0] = W_out[:, 0]
    w2[D:, 1] = W_out[:, 0]
    wh2t = np.ascontiguousarray(W_hid2[:H])
    wh2b = np.ascontiguousarray(W_hid2[H:])

    nc = _build_program(float(out_bias[0]))

    in_maps = []
    for c in range(NCORES):
        in_maps.append({
            "xt": xt,
            "xtm": np.ascontiguousarray(xt[:, c * R:(c + 1) * R]),
            "wfoh": W_foh,
            "wfom": W_fom,
            "cbh": cbh,
            "cbm": cbm,
            "h2b2": h2b2,
            "w2": w2,
            "wh2t": wh2t,
            "wh2b": wh2b,
        })

    res = run_bass_kernel_spmd(nc, in_maps, list(range(NCORES)))
    out = np.concatenate([res.results[c]["out"] for c in range(NCORES)],
                         axis=0)
    return out.astype(np.float32)


if __name__ == "__main__":
    rng = np.random.default_rng(0)
    ins = {
        "x": rng.standard_normal((N, 2, F // 2), dtype=np.float32),
        "W_foh": rng.standard_normal((F, H), dtype=np.float32) * 0.05,
        "W_fom": rng.standard_normal((F, H), dtype=np.float32) * 0.05,
        "cat_bias": rng.standard_normal((2 * H,), dtype=np.float32) * 0.05,
        "W_hid2": rng.standard_normal((2 * H, D), dtype=np.float32) * 0.05,
        "hid2_bias": rng.standard_normal((D,), dtype=np.float32) * 0.05,
        "W_out": rng.standard_normal((D, 1), dtype=np.float32) * 0.05,
        "out_bias": rng.standard_normal((1,), dtype=np.float32) * 0.05,
    }
    out = kernel(**ins)
    print("out", out.shape, out.dtype, out[:2, :4])


# revision 7
# speedup vs baseline: 1.1286x; 1.1286x over previous
"""Trainium2 Bass kernel for nn_ConcatHeadModule (pairwise MLP scores).

scores[i, j] = W_out . tanh(th[i] + tm[j] + hid2_bias) + out_bias
  th = tanh(xf @ W_foh + cat_bias[:H]) @ W_hid2[:H]
  tm = tanh(xf @ W_fom + cat_bias[H:]) @ W_hid2[H:]

Sharding: rows i split across 8 cores (128 rows each); everything else
replicated.

Device layout: hid2 (64) is stacked twice on SBUF partitions so one tanh
tile covers a pair of output rows (i, i+64). ACT fuses the per-pair th[i]
add via its per-partition bias operand and writes float32r (1 PE cycle/col).
The hid2 reduction runs on PE with a [128,16] stationary whose columns
one-hot route each pair's two output rows; 8 pairs accumulate into one
[16,1024] PSUM tile (zeros elsewhere), so the result sits dense on
partitions 0..15 and evacuates with a single cheap DVE op per group.
"""

import sys

sys.path.insert(0, "/opt/trn_rl_repo")

import numpy as np

import concourse.bass as bass
import concourse.tile as tile
from concourse import bacc, mybir
from concourse.bass_utils import run_bass_kernel_spmd

N = 1024          # nodes
F = 512           # 2 * LDIMS
H = 128           # hidden
D = 64            # hid2
NCORES = 8
R = N // NCORES   # rows per core = 128
NPAIR = R // 2    # row pairs per core = 64

F32 = mybir.dt.float32
F32R = mybir.dt.float32r
Tanh = mybir.ActivationFunctionType.Tanh

PAIRS_PER_GROUP = 8
NGROUPS = NPAIR // PAIRS_PER_GROUP


def _build_program(out_bias: float):
    nc = bacc.Bacc("TRN2", target_bir_lowering=False, debug=False,
                   num_devices=NCORES)

    xt_d = nc.dram_tensor("xt", [F, N], F32, kind="ExternalInput")
    xtm_d = nc.dram_tensor("xtm", [F, R], F32, kind="ExternalInput")
    wfoh_d = nc.dram_tensor("wfoh", [F, H], F32, kind="ExternalInput")
    wfom_d = nc.dram_tensor("wfom", [F, H], F32, kind="ExternalInput")
    cbh_d = nc.dram_tensor("cbh", [H, 1], F32, kind="ExternalInput")
    cbm_d = nc.dram_tensor("cbm", [H, 1], F32, kind="ExternalInput")
    h2bh_d = nc.dram_tensor("h2bh", [D, 1], F32, kind="ExternalInput")
    w2all_d = nc.dram_tensor("w2all", [2 * D, 16 * PAIRS_PER_GROUP], F32,
                             kind="ExternalInput")
    wh2t_d = nc.dram_tensor("wh2t", [H, D], F32, kind="ExternalInput")
    wh2b_d = nc.dram_tensor("wh2b", [H, D], F32, kind="ExternalInput")
    out_d = nc.dram_tensor("out", [R, N], F32, kind="ExternalOutput")

    with tile.TileContext(nc) as tc:
        with (
            tc.tile_pool(name="consts", bufs=1) as consts,
            tc.tile_pool(name="proj", bufs=1) as proj,
            tc.tile_pool(name="tanb", bufs=6) as tanp,
            tc.tile_pool(name="stage", bufs=2) as stagep,
            tc.tile_pool(name="ps", bufs=2, space="PSUM") as psum,
            tc.tile_pool(name="pscore", bufs=2, space="PSUM") as psump,
        ):
            # ---- load inputs, round matmul operands to f32r (DVE copy) ----
            def load_rounded(name, dram, shape):
                raw = consts.tile(shape, F32, tag=f"{name}_raw")
                nc.sync.dma_start(raw[:], dram)
                rnd = consts.tile(shape, F32R, tag=name)
                nc.vector.tensor_copy(rnd[:], raw[:])
                return rnd

            xtb = [load_rounded(f"xtb{q}", xt_d[q * H:(q + 1) * H, :], [H, N])
                   for q in range(4)]
            xtm = [load_rounded(f"xtm{q}", xtm_d[q * H:(q + 1) * H, :], [H, R])
                   for q in range(4)]
            wfoh = [load_rounded(f"wfoh{q}", wfoh_d[q * H:(q + 1) * H, :],
                                 [H, H]) for q in range(4)]
            wfom = [load_rounded(f"wfom{q}", wfom_d[q * H:(q + 1) * H, :],
                                 [H, H]) for q in range(4)]
            wh2t = load_rounded("wh2t", wh2t_d[:], [H, D])
            wh2b = load_rounded("wh2b", wh2b_d[:], [H, D])
            w2all = load_rounded("w2all", w2all_d[:],
                                 [2 * D, 16 * PAIRS_PER_GROUP])
            cbh = consts.tile([H, 1], F32, tag="cbh")
            nc.sync.dma_start(cbh[:], cbh_d[:])
            cbm = consts.tile([H, 1], F32, tag="cbm")
            nc.sync.dma_start(cbm[:], cbm_d[:])
            h2bh = consts.tile([D, 1], F32, tag="h2bh")
            nc.sync.dma_start(h2bh[:], h2bh_d[:])

            # ---- projections (all PE work in f32r, outputs at base 0) ----
            # modfovT over all nodes: tanh(W_fom^T @ xf^T + cbm)  [H, N]
            tanhm = proj.tile([H, N], F32R, tag="tanhm")
            for jh in range(2):
                pm = psum.tile([H, 512], F32, tag="ps")
                mv = slice(jh * 512, (jh + 1) * 512)
                for q in range(4):
                    nc.tensor.matmul(pm[:], wfom[q][:], xtb[q][:, mv],
                                     start=(q == 0), stop=(q == 3))
                nc.scalar.activation(tanhm[:, mv], pm[:], Tanh, bias=cbm[:])
            # headfovT for this core's rows: [H, R]
            tanhh = proj.tile([H, R], F32R, tag="tanhh")
            pm2 = psum.tile([H, R], F32, tag="ps")
            for q in range(4):
                nc.tensor.matmul(pm2[:], wfoh[q][:], xtm[q][:],
                                 start=(q == 0), stop=(q == 3))
            nc.scalar.activation(tanhh[:], pm2[:], Tanh, bias=cbh[:])

            # tmT + hid2_bias once at base 0, then DMA into both halves
            tm_half = proj.tile([D, N], F32, tag="tm_half")
            pt = psum.tile([D, N], F32, tag="ps")
            for jh in range(2):
                mv = slice(jh * 512, (jh + 1) * 512)
                nc.tensor.matmul(pt[:, mv], wh2b[:], tanhm[:, mv],
                                 start=True, stop=True)
            nc.vector.tensor_scalar_add(tm_half[:], pt[:], h2bh[:])
            tm_tile = proj.tile([2 * D, N], F32, tag="tm_tile")
            nc.sync.dma_start(tm_tile[0:D, :], tm_half[:])
            nc.sync.dma_start(tm_tile[D:2 * D, :], tm_half[:])

            # thT at base 0, then DMA the two row-halves into th_stack
            th_half = proj.tile([D, R], F32, tag="th_half")
            ps3 = psum.tile([D, R], F32, tag="ps")
            nc.tensor.matmul(ps3[:], wh2t[:], tanhh[:], start=True, stop=True)
            nc.vector.tensor_copy(th_half[:], ps3[:])
            th_stack = proj.tile([2 * D, NPAIR], F32, tag="th_stack")
            nc.sync.dma_start(th_stack[0:D, :], th_half[:, 0:NPAIR])
            nc.sync.dma_start(th_stack[D:2 * D, :], th_half[:, NPAIR:R])

            # ---- main pair loop ----
            # group g covers pairs p = 8g+u -> rows {8g+u, 64+8g+u}.
            # PSUM row u = local row 8g+u (w=0), row 8+u = 64+8g+u (w=1).
            for g in range(NGROUPS):
                pscore = psump.tile([16, N], F32, tag="pscore")
                for u in range(PAIRS_PER_GROUP):
                    p = g * PAIRS_PER_GROUP + u
                    tanb = tanp.tile([2 * D, N], F32R, tag="tanb")
                    nc.scalar.activation(tanb[:], tm_tile[:], Tanh,
                                         bias=th_stack[:, p:p + 1])
                    for jh in range(2):
                        mv = slice(jh * 512, (jh + 1) * 512)
                        nc.tensor.matmul(
                            pscore[:, mv], w2all[:, 16 * u:16 * (u + 1)],
                            tanb[:, mv],
                            start=(u == 0), stop=(u == PAIRS_PER_GROUP - 1),
                            skip_group_check=True)
                stg = stagep.tile([16, N], F32, tag="stg")
                nc.vector.tensor_scalar_add(stg[:], pscore[:], out_bias)
                base = g * PAIRS_PER_GROUP
                nc.sync.dma_start(out_d[base:base + 8, :], stg[0:8, :])
                nc.sync.dma_start(out_d[64 + base:64 + base + 8, :],
                                  stg[8:16, :])

    nc.compile()
    return nc


def _make_in_maps(x, W_foh, W_fom, cat_bias, W_hid2, hid2_bias, W_out):
    xf = x.reshape(N, F)
    xt = np.ascontiguousarray(xf.T)                      # [F, N]
    cbh = np.ascontiguousarray(cat_bias[:H].reshape(H, 1))
    cbm = np.ascontiguousarray(cat_bias[H:].reshape(H, 1))
    h2bh = np.ascontiguousarray(hid2_bias.reshape(D, 1))
    # w2all[:, 16u + c]: c==u -> [W_out; 0] (row 8g+u), c==8+u -> [0; W_out]
    w2all = np.zeros((2 * D, 16 * PAIRS_PER_GROUP), dtype=np.float32)
    for u in range(PAIRS_PER_GROUP):
        w2all[:D, 16 * u + u] = W_out[:, 0]
        w2all[D:, 16 * u + 8 + u] = W_out[:, 0]
    wh2t = np.ascontiguousarray(W_hid2[:H])
    wh2b = np.ascontiguousarray(W_hid2[H:])
    in_maps = []
    for c in range(NCORES):
        in_maps.append({
            "xt": xt,
            "xtm": np.ascontiguousarray(xt[:, c * R:(c + 1) * R]),
            "wfoh": W_foh,
            "wfom": W_fom,
            "cbh": cbh,
            "cbm": cbm,
            "h2bh": h2bh,
            "w2all": w2all,
            "wh2t": wh2t,
            "wh2b": wh2b,
        })
    return in_maps


def kernel(x, W_foh, W_fom, cat_bias, W_hid2, hid2_bias, W_out, out_bias):
    x = np.asarray(x, dtype=np.float32)
    W_foh = np.asarray(W_foh, dtype=np.float32)
    W_fom = np.asarray(W_fom, dtype=np.float32)
    cat_bias = np.asarray(cat_bias, dtype=np.float32)
    W_hid2 = np.asarray(W_hid2, dtype=np.float32)
    hid2_bias = np.asarray(hid2_bias, dtype=np.float32)
    W_out = np.asarray(W_out, dtype=np.float32)
    out_bias = np.asarray(out_bias, dtype=np.float32)

    nc = _build_program(float(out_bias[0]))
    in_maps = _make_in_maps(x, W_foh, W_fom, cat_bias, W_hid2, hid2_bias,
                            W_out)
    res = run_bass_kernel_spmd(nc, in_maps, list(range(NCORES)))
    out = np.concatenate([res.results[c]["out"] for c in range(NCORES)],
                         axis=0)
    return out.astype(np.float32)


if __name__ == "__main__":
    rng = np.random.default_rng(0)
    ins = {
        "x": rng.standard_normal((N, 2, F // 2), dtype=np.float32),
        "W_foh": rng.standard_normal((F, H), dtype=np.float32) * 0.05,
        "W_fom": rng.standard_normal((F, H), dtype=np.float32) * 0.05,
        "cat_bias": rng.standard_normal((2 * H,), dtype=np.float32) * 0.05,
        "W_hid2": rng.standard_normal((2 * H, D), dtype=np.float32) * 0.05,
        "hid2_bias": rng.standard_normal((D,), dtype=np.float32) * 0.05,
        "W_out": rng.standard_normal((D, 1), dtype=np.float32) * 0.05,
        "out_bias": rng.standard_normal((1,), dtype=np.float32) * 0.05,
    }
    out = kernel(**ins)
    print("out", out.shape, out.dtype, out[:2, :4])


# revision 10
# speedup vs baseline: 1.1742x; 1.0404x over previous
"""Trainium2 Bass kernel for nn_ConcatHeadModule (pairwise MLP scores).

scores[i, j] = W_out . tanh(th[i] + tm[j] + hid2_bias) + out_bias
  th = tanh(xf @ W_foh + cat_bias[:H]) @ W_hid2[:H]
  tm = tanh(xf @ W_fom + cat_bias[H:]) @ W_hid2[H:]

Sharding: rows i split across 8 cores (128 rows each); everything else
replicated.

Device layout: hid2 (64) is stacked twice on SBUF partitions so one tanh
tile covers a pair of output rows (i, i+64). ACT fuses the per-pair th[i]
add via its per-partition bias operand and writes float32r (1 PE cycle/col).
The hid2 reduction runs on PE with a [128,16] stationary whose columns
one-hot route each pair's two output rows; 8 pairs accumulate into one
[16,1024] PSUM tile (zeros elsewhere), so the result sits dense on
partitions 0..15 and evacuates with a single cheap DVE op per group.
"""

import sys

sys.path.insert(0, "/opt/trn_rl_repo")

import numpy as np

import concourse.bass as bass
import concourse.tile as tile
from concourse import bacc, mybir
from concourse.bass_utils import run_bass_kernel_spmd

N = 1024          # nodes
F = 512           # 2 * LDIMS
H = 128           # hidden
D = 64            # hid2
NCORES = 8
R = N // NCORES   # rows per core = 128
NPAIR = R // 2    # row pairs per core = 64

F32 = mybir.dt.float32
F32R = mybir.dt.float32r
Tanh = mybir.ActivationFunctionType.Tanh

PAIRS_PER_GROUP = 8
NGROUPS = NPAIR // PAIRS_PER_GROUP


def _build_program(out_bias: float):
    nc = bacc.Bacc("TRN2", target_bir_lowering=False, debug=False,
                   num_devices=NCORES)

    xt_d = nc.dram_tensor("xt", [F, N], F32, kind="ExternalInput")
    xtm_d = nc.dram_tensor("xtm", [F, R], F32, kind="ExternalInput")
    wfoh_d = nc.dram_tensor("wfoh", [F, H], F32, kind="ExternalInput")
    wfom_d = nc.dram_tensor("wfom", [F, H], F32, kind="ExternalInput")
    cbh_d = nc.dram_tensor("cbh", [H, 1], F32, kind="ExternalInput")
    cbm_d = nc.dram_tensor("cbm", [H, 1], F32, kind="ExternalInput")
    h2bh_d = nc.dram_tensor("h2bh", [D, 1], F32, kind="ExternalInput")
    w2all_d = nc.dram_tensor("w2all", [2 * D, 16 * PAIRS_PER_GROUP], F32,
                             kind="ExternalInput")
    wh2t_d = nc.dram_tensor("wh2t", [H, D], F32, kind="ExternalInput")
    wh2b_d = nc.dram_tensor("wh2b", [H, D], F32, kind="ExternalInput")
    out_d = nc.dram_tensor("out", [R, N], F32, kind="ExternalOutput")

    with tile.TileContext(nc) as tc:
        with (
            tc.tile_pool(name="consts", bufs=1) as consts,
            tc.tile_pool(name="proj", bufs=1) as proj,
            tc.tile_pool(name="tanb", bufs=6) as tanp,
            tc.tile_pool(name="stage", bufs=2) as stagep,
            tc.tile_pool(name="ps", bufs=2, space="PSUM") as psum,
            tc.tile_pool(name="pscore", bufs=2, space="PSUM") as psump,
        ):
            # ---- load inputs, round matmul operands to f32r (DVE copy) ----
            # Trigger the tanh ACT table load immediately (overlaps loads).
            warm = consts.tile([H, 1], F32, tag="warm")
            nc.vector.memset(warm[:], 0.0)
            nc.scalar.activation(warm[:], warm[:], Tanh)

            # Round-robin DMA loads over engine queues so transfers overlap.
            _engs = [nc.sync, nc.gpsimd]
            _eng_i = [0]

            def _dma(dst, src):
                e = _engs[_eng_i[0] % len(_engs)]
                _eng_i[0] += 1
                e.dma_start(dst, src)

            def load_rounded(name, dram, shape):
                raw = consts.tile(shape, F32, tag=f"{name}_raw")
                _dma(raw[:], dram)
                rnd = consts.tile(shape, F32R, tag=name)
                nc.vector.tensor_copy(rnd[:], raw[:])
                return rnd

            xtb = [load_rounded(f"xtb{q}", xt_d[q * H:(q + 1) * H, :], [H, N])
                   for q in range(4)]
            xtm = [load_rounded(f"xtm{q}", xtm_d[q * H:(q + 1) * H, :], [H, R])
                   for q in range(4)]
            wfoh = [load_rounded(f"wfoh{q}", wfoh_d[q * H:(q + 1) * H, :],
                                 [H, H]) for q in range(4)]
            wfom = [load_rounded(f"wfom{q}", wfom_d[q * H:(q + 1) * H, :],
                                 [H, H]) for q in range(4)]
            wh2t = load_rounded("wh2t", wh2t_d[:], [H, D])
            wh2b = load_rounded("wh2b", wh2b_d[:], [H, D])
            w2all = load_rounded("w2all", w2all_d[:],
                                 [2 * D, 16 * PAIRS_PER_GROUP])
            cbh = consts.tile([H, 1], F32, tag="cbh")
            _dma(cbh[:], cbh_d[:])
            cbm = consts.tile([H, 1], F32, tag="cbm")
            _dma(cbm[:], cbm_d[:])
            h2bh = consts.tile([D, 1], F32, tag="h2bh")
            _dma(h2bh[:], h2bh_d[:])

            # ---- projections (all PE work in f32r, outputs at base 0) ----
            # modfovT over all nodes: tanh(W_fom^T @ xf^T + cbm)  [H, N]
            tanhm = proj.tile([H, N], F32R, tag="tanhm")
            for jh in range(2):
                pm = psum.tile([H, 512], F32, tag="ps")
                mv = slice(jh * 512, (jh + 1) * 512)
                for q in range(4):
                    nc.tensor.matmul(pm[:], wfom[q][:], xtb[q][:, mv],
                                     start=(q == 0), stop=(q == 3))
                nc.scalar.activation(tanhm[:, mv], pm[:], Tanh, bias=cbm[:])
            # headfovT for this core's rows: [H, R]
            tanhh = proj.tile([H, R], F32R, tag="tanhh")
            pm2 = psum.tile([H, R], F32, tag="ps")
            for q in range(4):
                nc.tensor.matmul(pm2[:], wfoh[q][:], xtm[q][:],
                                 start=(q == 0), stop=(q == 3))
            nc.scalar.activation(tanhh[:], pm2[:], Tanh, bias=cbh[:])

            # tmT + hid2_bias once at base 0, then DMA into both halves
            tm_half = proj.tile([D, N], F32, tag="tm_half")
            pt = psum.tile([D, N], F32, tag="ps")
            for jh in range(2):
                mv = slice(jh * 512, (jh + 1) * 512)
                nc.tensor.matmul(pt[:, mv], wh2b[:], tanhm[:, mv],
                                 start=True, stop=True)
            nc.vector.tensor_scalar_add(tm_half[:], pt[:], h2bh[:])
            tm_tile = proj.tile([2 * D, N], F32, tag="tm_tile")
            nc.sync.dma_start(tm_tile[0:D, :], tm_half[:])
            nc.gpsimd.dma_start(tm_tile[D:2 * D, :], tm_half[:])

            # thT at base 0, then DMA the two row-halves into th_stack
            th_half = proj.tile([D, R], F32, tag="th_half")
            ps3 = psum.tile([D, R], F32, tag="ps")
            nc.tensor.matmul(ps3[:], wh2t[:], tanhh[:], start=True, stop=True)
            nc.vector.tensor_copy(th_half[:], ps3[:])
            th_stack = proj.tile([2 * D, NPAIR], F32, tag="th_stack")
            nc.sync.dma_start(th_stack[0:D, :], th_half[:, 0:NPAIR])
            nc.gpsimd.dma_start(th_stack[D:2 * D, :], th_half[:, NPAIR:R])

            # ---- main pair loop ----
            # group g covers pairs p = 8g+u -> rows {8g+u, 64+8g+u}.
            # PSUM row u = local row 8g+u (w=0), row 8+u = 64+8g+u (w=1).
            for g in range(NGROUPS):
                pscore = psump.tile([16, N], F32, tag="pscore")
                for u in range(PAIRS_PER_GROUP):
                    p = g * PAIRS_PER_GROUP + u
                    tanb = tanp.tile([2 * D, N], F32R, tag="tanb")
                    nc.scalar.activation(tanb[:], tm_tile[:], Tanh,
                                         bias=th_stack[:, p:p + 1])
                    for jh in range(2):
                        mv = slice(jh * 512, (jh + 1) * 512)
                        nc.tensor.matmul(
                            pscore[:, mv], w2all[:, 16 * u:16 * (u + 1)],
                            tanb[:, mv],
                            start=(u == 0), stop=(u == PAIRS_PER_GROUP - 1),
                            skip_group_check=True)
                stg = stagep.tile([16, N], F32, tag="stg")
                nc.vector.tensor_scalar_add(stg[:], pscore[:], out_bias)
                base = g * PAIRS_PER_GROUP
                nc.sync.dma_start(out_d[base:base + 8, :], stg[0:8, :])
                nc.sync.dma_start(out_d[64 + base:64 + base + 8, :],
                                  stg[8:16, :])

    nc.compile()
    return nc


def _make_in_maps(x, W_foh, W_fom, cat_bias, W_hid2, hid2_bias, W_out):
    xf = x.reshape(N, F)
    xt = np.ascontiguousarray(xf.T)                      # [F, N]
    cbh = np.ascontiguousarray(cat_bias[:H].reshape(H, 1))
    cbm = np.ascontiguousarray(cat_bias[H:].reshape(H, 1))
    h2bh = np.ascontiguousarray(hid2_bias.reshape(D, 1))
    # w2all[:, 16u + c]: c==u -> [W_out; 0] (row 8g+u), c==8+u -> [0; W_out]
    w2all = np.zeros((2 * D, 16 * PAIRS_PER_GROUP), dtype=np.float32)
    for u in range(PAIRS_PER_GROUP):
        w2all[:D, 16 * u + u] = W_out[:, 0]
        w2all[D:, 16 * u + 8 + u] = W_out[:, 0]
    wh2t = np.ascontiguousarray(W_hid2[:H])
    wh2b = np.ascontiguousarray(W_hid2[H:])
    in_maps = []
    for c in range(NCORES):
        in_maps.append({
            "xt": xt,
            "xtm": np.ascontiguousarray(xt[:, c * R:(c + 1) * R]),
            "wfoh": W_foh,
            "wfom": W_fom,
            "cbh": cbh,
            "cbm": cbm,
            "h2bh": h2bh,
            "w2all": w2all,
            "wh2t": wh2t,
            "wh2b": wh2b,
        })
    return in_maps


def kernel(x, W_foh, W_fom, cat_bias, W_hid2, hid2_bias, W_out, out_bias):
    x = np.asarray(x, dtype=np.float32)
    W_foh = np.asarray(W_foh, dtype=np.float32)
    W_fom = np.asarray(W_fom, dtype=np.float32)
    cat_bias = np.asarray(cat_bias, dtype=np.float32)
    W_hid2 = np.asarray(W_hid2, dtype=np.float32)
    hid2_bias = np.asarray(hid2_bias, dtype=np.float32)
    W_out = np.asarray(W_out, dtype=np.float32)
    out_bias = np.asarray(out_bias, dtype=np.float32)

    nc = _build_program(float(out_bias[0]))
    in_maps = _make_in_maps(x, W_foh, W_fom, cat_bias, W_hid2, hid2_bias,
                            W_out)
    res = run_bass_kernel_spmd(nc, in_maps, list(range(NCORES)))
    out = np.concatenate([res.results[c]["out"] for c in range(NCORES)],
                         axis=0)
    return out.astype(np.float32)


if __name__ == "__main__":
    rng = np.random.default_rng(0)
    ins = {
        "x": rng.standard_normal((N, 2, F // 2), dtype=np.float32),
        "W_foh": rng.standard_normal((F, H), dtype=np.float32) * 0.05,
        "W_fom": rng.standard_normal((F, H), dtype=np.float32) * 0.05,
        "cat_bias": rng.standard_normal((2 * H,), dtype=np.float32) * 0.05,
        "W_hid2": rng.standard_normal((2 * H, D), dtype=np.float32) * 0.05,
        "hid2_bias": rng.standard_normal((D,), dtype=np.float32) * 0.05,
        "W_out": rng.standard_normal((D, 1), dtype=np.float32) * 0.05,
        "out_bias": rng.standard_normal((1,), dtype=np.float32) * 0.05,
    }
    out = kernel(**ins)
    print("out", out.shape, out.dtype, out[:2, :4])
